# revision 14
# baseline (speedup 1.0000x reference)
"""2D Haar DWT (DWT_2D) Trainium2 Bass kernel.

Input:  input [8, 64, 512, 512] f32 plus the four Haar DWT matrices.
Output: (LL, LH, HL, HH), each [8, 64, 256, 256] f32.

The Haar matrices have exactly two nonzeros (+-1/sqrt(2)) per row/col, so the
whole DWT is a 2x2 butterfly per input block:
    LL = 0.5*(a+b+c+d), LH = 0.5*(a-b+c-d),
    HL = 0.5*(a+b-c-d), HH = 0.5*(a-b-c+d)
with a=x[2i,2j], b=x[2i,2j+1], c=x[2i+1,2j], d=x[2i+1,2j+1]. The 0.5 scale is
folded into the host-side shard copy, and the reference's last-row/last-col
zero quirks (Hh row 255, mh1 col 255) are applied on the host after the
gather — the device does pure adds/subs plus DMA.

The f32 version of this kernel measured 353 GB/s aggregate DMA — the per-core
HBM share is the wall (DMA active 95.9% of exec time). So the device kernel
runs in bf16: the host folds the 0.5 scale and casts the input shard to bf16
(one rounding, ~0.2% rel err, far inside the 2e-2 gate), the device
butterflies in bf16 (2x DVE throughput), and outputs land in bf16 and are
upcast on the host. HBM traffic halves: 67 MB/core -> ~190 us at 353 GB/s.

Sharding: data-parallel over the batch dim, one batch element (64 slices of
[512,512]) per NeuronCore. Device kernel processes 4 slices per iteration:
one contiguous 2MB in-DMA (16KB/partition lines), vertical butterfly on DVE,
horizontal butterflies split DVE/GpSimd, two 1MB out-DMAs (4KB runs).
"""

import math
import os

import numpy as np

import concourse.bacc as bacc
import concourse.bass as bass
import concourse.mybir as mybir
from concourse.alu_op_type import AluOpType
from concourse.bass_utils import run_bass_kernel_spmd
from concourse.tile import TileContext

B, C, H, W = 8, 64, 512, 512
N_CORES = 8
SLICES_PER_CORE = (B * C) // N_CORES  # 64 [512,512] slices per core
GROUP = 4  # slices per device iteration
FP = mybir.dt.bfloat16

_prog_cache = {}

# Set by test/profiling harnesses: when True, run_bass_kernel_spmd captures an
# NTFF profile and the BassKernelResults lands in LAST_RESULTS.
TRACE = False
LAST_RESULTS = None


def _bf16():
    import ml_dtypes

    return ml_dtypes.bfloat16


def _build_program(n_slices: int) -> bass.Bass:
    # Bacc (not raw Bass): its compile() pass converts the Tile exit drain's
    # many sem waits into event semaphores; raw Bass fails walrus codegen
    # with "Too many sync wait commands".
    nc = bacc.Bacc(None, target_bir_lowering=False)
    x = nc.dram_tensor("x", [n_slices, H, W], FP, kind="ExternalInput")
    # Band-interleaved output: [slice, band_row, band, 256]. Interleaving the
    # four bands per row makes each partition's out-DMA payload one
    # contiguous 16KB DRAM run (vs 4KB band-major runs, which capped the
    # out stream at 245 GB/s). The host unpacks to band-major after gather.
    out = nc.dram_tensor(
        "out", [n_slices, H // 2, 4, W // 2], FP, kind="ExternalOutput"
    )

    n_iter = n_slices // GROUP
    # Input: 4 slices = 2048 rows; partition p holds rows 16p..16p+15, i.e.
    # rows 16pp..16pp+15 of slice a for p = 32a+pp. One contiguous 2MB DMA,
    # 16KB per partition line.
    x2 = x[:].rearrange("(i a) h w -> i (a h) w", a=GROUP)  # [i, 2048, 512]
    # Output: partition p holds band rows 8pp..8pp+7 (all 4 bands) of slice
    # GROUP*i + a: one contiguous 16KB run per partition per iteration.
    ov = out[:].rearrange(
        "(i a) (pp t) b w -> i (a pp) t b w", a=GROUP, t=8
    )  # [i, 128, 8, 4, 256]

    with TileContext(nc) as tc:
        with tc.tile_pool(name="pool", bufs=4) as pool:
            for i in range(n_iter):
                xt = pool.tile([128, 16, 512], FP, tag="xt", bufs=3)
                # In-DMAs on the Sync sequencer; out-DMAs on the (otherwise
                # idle) Scalar sequencer so out-DMA waits can't
                # head-of-line-block in-DMA issue. (Measured dead ends:
                # splitting the in-DMA/deint into row-halves — globally or
                # just for iteration 0 — regressed 20-35us; the non-uniform
                # iteration bodies perturb Tile's schedule well beyond the
                # intended ramp saving.)
                nc.sync.dma_start(
                    out=xt[:], in_=x2[i].rearrange("(p q) w -> p q w", p=128)
                )

                # ACT deinterleaves the column pairs once (1x-rate strided
                # copy on its own datapath/SBUF port): xd[p,q,t,j] =
                # xt[p,q,2j+t]. After that EVERY butterfly op is a stride-1
                # bf16 tensor_tensor on DVE in 2x mode (8192 cyc/iter vs
                # 12288 for the reduce-based split), and GpSimd stays fully
                # idle — it shares its physical SBUF port with DVE (the POOL
                # slot), so any GpSimd work slows DVE ops ~2x (measured).
                xd = pool.tile([128, 16, 2, 256], FP, tag="xd", bufs=3)
                nc.scalar.activation(
                    out=xd[:],
                    in_=xt[:].rearrange("p q (j t) -> p q t j", t=2),
                    func=mybir.ActivationFunctionType.Copy,
                )

                xe = xd[:, 0:16:2]  # even rows of the eight pairs
                xo = xd[:, 1:16:2]  # odd rows
                st = pool.tile([128, 8, 2, 256], FP, tag="st", bufs=3)
                dt = pool.tile([128, 8, 2, 256], FP, tag="dt", bufs=3)
                nc.vector.tensor_add(out=st[:], in0=xe, in1=xo)
                nc.vector.tensor_sub(out=dt[:], in0=xe, in1=xo)

                # Horizontal butterflies: stride-1 [128, 8, 256] TTs writing
                # the band-interleaved om tile (innermost dim contiguous, so
                # 2x mode holds). Everything yields 2*(LL,LH,HL,HH); the
                # host applies the 0.5 DWT scale after the gather.
                om = pool.tile([128, 8, 4, 256], FP, tag="om", bufs=3)
                nc.vector.tensor_add(out=om[:, :, 0], in0=st[:, :, 0], in1=st[:, :, 1])
                nc.vector.tensor_sub(out=om[:, :, 1], in0=st[:, :, 0], in1=st[:, :, 1])
                nc.vector.tensor_add(out=om[:, :, 2], in0=dt[:, :, 0], in1=dt[:, :, 1])
                nc.vector.tensor_sub(out=om[:, :, 3], in0=dt[:, :, 0], in1=dt[:, :, 1])

                nc.scalar.dma_start(out=ov[i], in_=om[:])
    nc.finalize()
    return nc


def _get_program(n_slices: int) -> bass.Bass:
    if n_slices not in _prog_cache:
        _prog_cache[n_slices] = _build_program(n_slices)
    return _prog_cache[n_slices]


def _expected_matrices():
    """Numpy port of reference.build_dwt_matrices for Haar, H=W=512."""
    sq = 1.0 / math.sqrt(2.0)
    ml0 = np.zeros((256, 512), np.float32)
    mh0 = np.zeros((256, 512), np.float32)
    for i in range(256):
        ml0[i, 2 * i : 2 * i + 2] = [sq, sq]
    for i in range(255):  # last row left zero (reference quirk)
        mh0[i, 2 * i : 2 * i + 2] = [sq, -sq]
    return ml0, ml0.T.copy(), mh0, mh0.T.copy()


def _numpy_fallback(x, ml0, ml1, mh0, mh1):
    out = []
    l = np.einsum("ih,bchw->bciw", ml0, x, optimize=True)
    hh_ = np.einsum("ih,bchw->bciw", mh0, x, optimize=True)
    for m in (l, hh_):
        for right in (ml1, mh1):
            out.append(np.einsum("bciw,wj->bcij", m, right, optimize=True))
    return tuple(np.ascontiguousarray(o.astype(np.float32)) for o in out)


def kernel(**inputs):
    x = np.asarray(inputs["input"], dtype=np.float32)
    assert x.shape == (B, C, H, W), x.shape

    ml0 = np.asarray(inputs["matrix_low_0"], dtype=np.float32)
    ml1 = np.asarray(inputs["matrix_low_1"], dtype=np.float32)
    mh0 = np.asarray(inputs["matrix_high_0"], dtype=np.float32)
    mh1 = np.asarray(inputs["matrix_high_1"], dtype=np.float32)
    el0, el1, eh0, eh1 = _expected_matrices()
    if not (
        np.array_equal(ml0, el0)
        and np.array_equal(ml1, el1)
        and np.array_equal(mh0, eh0)
        and np.array_equal(mh1, eh1)
    ):
        # Unexpected (non-Haar) matrices: stay correct via numpy.
        return _numpy_fallback(x, ml0, ml1, mh0, mh1)

    bf16 = _bf16()
    nc = _get_program(SLICES_PER_CORE)
    xs = x.reshape(B * C, H, W)
    # No pre-scale: the device computes 2*(LL,LH,HL,HH) and the 0.5 DWT
    # scale is applied on the host after the gather (exact in fp32).
    in_maps = [
        {"x": xs[i * SLICES_PER_CORE : (i + 1) * SLICES_PER_CORE].astype(bf16)}
        for i in range(N_CORES)
    ]
    global LAST_RESULTS
    try:
        res = run_bass_kernel_spmd(
            nc, in_maps, core_ids=list(range(N_CORES)), trace=TRACE
        )
    except ModuleNotFoundError:
        # A stray BASS_TRACE=1 in the environment routes through the NTFF
        # hook import, which this image lacks — retry untraced.
        os.environ["BASS_NEVER_TRACE"] = "1"
        res = run_bass_kernel_spmd(
            nc, in_maps, core_ids=list(range(N_CORES)), trace=False
        )
    LAST_RESULTS = res
    # Device layout: [slice, band_row, band, 256] per core; unpack to
    # band-major [4, B, C, 256, 256] on the host.
    full = np.ascontiguousarray(
        np.concatenate([res.results[i]["out"] for i in range(N_CORES)], axis=0)
        .transpose(2, 0, 1, 3)
    ).astype(np.float32).reshape(4, B, C, H // 2, W // 2)
    full *= 0.5  # device computed 2*(LL,LH,HL,HH)
    ll, lh, hl, hh = full[0], full[1], full[2], full[3]
    # Reference quirks: Hh row 255 == 0 (HL/HH row 255), mh1 col 255 == 0
    # (LH/HH col 255).
    lh[..., :, 255] = 0.0
    hl[..., 255, :] = 0.0
    hh[..., 255, :] = 0.0
    hh[..., :, 255] = 0.0
    return (ll, lh, hl, hh)

